# revision 1
# baseline (speedup 1.0000x reference)
"""Trainium2 Bass kernel for nn_CameraAwareSparseBlock (sparse submanifold 3x3x3
conv x2 + BN + FiLM + residual), data-parallel over 8 NeuronCores.

Strategy (per core, SPMD):
  - Each core owns N/8 consecutive voxels (voxels are sorted by batch-major key,
    so neighbor indices stay within +-margin positions of each output row).
  - Features are kept TRANSPOSED and bf16 channel-PAIR-packed in uint32 words:
    source layout [pair_channels, voxels]. GPSIMD ap_gather gathers arbitrary
    voxel columns for 8 taps (conv1, 16 pair-rows/tap) or 4 taps (conv2,
    32 pair-rows/tap) per instruction into a [128, 512] tile that feeds the
    TensorEngine directly: two matmuls per gather (even/odd bf16 halves via
    stride-2 access patterns) contract K=128 = taps x pair-channels.
  - conv1 (P-A) streams windowed source chunks, accumulates BN stats from f32
    PSUM, stages pre-BN outputs (even/odd channel split) to HBM.
  - AllReduce (sum, sumsq) across 8 cores -> BN1 affine coeffs.
  - conv2 (P-C) reloads staged outputs with 4x partition replication, applies
    BN1 affine + ReLU while pair-packing, gathers + matmuls, stages pre-BN2
    outputs, accumulates BN2 stats.
  - AllReduce -> BN2 + FiLM coeffs (camera MLP computed on-device).
  - Epilogue (P-D): affine+ReLU, adds the 1x1 residual (matmul from the packed
    feature source), transposes via PE, writes [N/8, 64] f32 rows.
"""

import numpy as np
import ml_dtypes
from contextlib import ExitStack
from dataclasses import dataclass

import concourse.bass as bass
import concourse.tile as tile
from concourse import bacc, mybir
from concourse.bass_utils import run_bass_kernel_spmd
from concourse.masks import make_identity

BF16 = ml_dtypes.bfloat16
F32 = np.float32
AF = mybir.ActivationFunctionType
ALU = mybir.AluOpType
AX = mybir.AxisListType

EPS = 1e-5


@dataclass(frozen=True)
class Geo:
    n: int          # total voxels
    n_cores: int    # 8
    til: int        # 512
    m: int          # neighbor position margin (multiple of til)
    at: int         # tiles per P-A chunk
    nch_a: int      # P-A chunks
    ct: int         # tiles per P-C chunk
    nch_c: int      # P-C chunks
    c1: int = 32
    c2: int = 64
    k: int = 27
    cam: int = 256

    @property
    def own(self):
        return self.n // self.n_cores

    @property
    def a_chunk(self):
        return self.at * self.til

    @property
    def c_chunk(self):
        return self.ct * self.til

    @property
    def win(self):
        return self.nch_a * self.a_chunk

    @property
    def own_p(self):
        return self.nch_c * self.c_chunk

    @property
    def src_a(self):
        return self.a_chunk + 2 * self.m + 1

    @property
    def src_c(self):
        return self.c_chunk + 2 * self.m + 1

    @property
    def fwinp(self):
        return (self.nch_a - 1) * self.a_chunk + self.src_a

    @property
    def winp(self):
        return max(self.win, (self.nch_c - 1) * self.c_chunk + self.src_c)

    def check(self):
        assert self.m % self.til == 0
        assert self.win >= self.own + 2 * self.m
        assert self.own_p >= self.own and self.own_p - self.own < self.til
        assert self.src_a <= 32768 and self.src_c <= 32768
        assert self.n % self.n_cores == 0


GEO_FULL = Geo(n=200000, n_cores=8, til=512, m=1024, at=8, nch_a=7, ct=7, nch_c=7)

PERM1 = np.array([2 * i for i in range(32)] + [2 * i + 1 for i in range(32)])

# conv1: 4 gather insts per tile covering taps [0:8),[8:16),[16:24),[24:27)
G1_TAPS = [8, 8, 8, 3]
# conv2: 7 gather insts covering 4,4,4,4,4,4,3 taps
G2_TAPS = [4, 4, 4, 4, 4, 4, 3]


# ---------------------------------------------------------------------------
# Device program
# ---------------------------------------------------------------------------

def build_module(g: Geo, single: bool = False):
    """single=True builds a 1-core variant with AllReduce replaced by a copy,
    for TimelineSim cost-model analysis (collectives unsupported there)."""
    g.check()
    nc = bacc.Bacc("TRN2", target_bir_lowering=False, debug=False,
                   num_devices=(1 if single else g.n_cores))
    u32, i16, bf, f32 = (mybir.dt.uint32, mybir.dt.int16,
                         mybir.dt.bfloat16, mybir.dt.float32)
    til = g.til
    ncol = til // 16

    # ---- I/O tensors (per core) ----
    # fp: host-replicated pair-packed features [128 = 8 reps x 16 pairs, fwinp]
    fp = nc.dram_tensor("fp", [128, g.fwinp], u32, kind="ExternalInput")
    idx1 = nc.dram_tensor("idx1", [128, g.nch_a * g.at * 4 * ncol], i16,
                          kind="ExternalInput")
    idx2 = nc.dram_tensor("idx2", [128, g.nch_c * g.ct * 7 * ncol], i16,
                          kind="ExternalInput")
    wpk1 = nc.dram_tensor("wpk1", [128, 8 * 64], bf, kind="ExternalInput")
    wpk2 = nc.dram_tensor("wpk2", [128, 14 * 64], bf, kind="ExternalInput")
    wrt = nc.dram_tensor("wrt", [17, 128], bf, kind="ExternalInput")
    wcs = nc.dram_tensor("wcs", [g.cam + 1, 64], f32, kind="ExternalInput")
    wcsh = nc.dram_tensor("wcsh", [g.cam + 1, 64], f32, kind="ExternalInput")
    camt = nc.dram_tensor("camt", [g.cam + 1, 4], f32, kind="ExternalInput")
    bsel = nc.dram_tensor("bsel", [64, 4], f32, kind="ExternalInput")
    gb1 = nc.dram_tensor("gb1", [64, 2], f32, kind="ExternalInput")  # g1p|be1p
    gb2 = nc.dram_tensor("gb2", [64, 2], f32, kind="ExternalInput")  # g2|be2
    brv = nc.dram_tensor("brv", [64, 1], f32, kind="ExternalInput")
    y = nc.dram_tensor("y", [g.own_p, 64], f32, kind="ExternalOutput")

    # ---- internal DRAM ----
    # conv1 pre-BN output, rows 0:32 = even channels (PERM1), 32:64 = odd
    hh = nc.dram_tensor("hh", [64, g.winp], bf, kind="Internal")
    h2pre = nc.dram_tensor("h2pre", [64, g.own_p], bf, kind="Internal")
    ar1i = nc.dram_tensor("ar1i", [64, 2], f32, kind="Internal")
    ar1o = nc.dram_tensor("ar1o", [64, 2], f32, kind="Internal")
    ar2i = nc.dram_tensor("ar2i", [64, 2], f32, kind="Internal")
    ar2o = nc.dram_tensor("ar2o", [64, 2], f32, kind="Internal")
    c1d = nc.dram_tensor("c1d", [64, 2], f32, kind="Internal")

    groups = [list(range(g.n_cores))]
    inv_n = 1.0 / float(g.n)

    with tile.TileContext(nc) as tc:
        with ExitStack() as ctx:
            cpool = ctx.enter_context(tc.tile_pool(name="const", bufs=1))
            bigp = ctx.enter_context(tc.tile_pool(name="bigsrc", bufs=2))
            gatp = ctx.enter_context(tc.tile_pool(name="gat", bufs=9))
            idxp = ctx.enter_context(tc.tile_pool(name="idx", bufs=2))
            stgp = ctx.enter_context(tc.tile_pool(name="stg", bufs=3))
            ldp = ctx.enter_context(tc.tile_pool(name="ld", bufs=3))
            psp = ctx.enter_context(tc.tile_pool(name="ps", bufs=6, space="PSUM"))
            ptp = ctx.enter_context(tc.tile_pool(name="pst", bufs=2, space="PSUM"))

            # ---- constants ----
            w1sb = cpool.tile([128, 8 * 64], bf)
            nc.sync.dma_start(w1sb[:], wpk1.ap())
            w2sb = cpool.tile([128, 14 * 64], bf)
            nc.sync.dma_start(w2sb[:], wpk2.ap())
            wrsb = cpool.tile([16, 128], bf)
            nc.sync.dma_start(wrsb[:], wrt.ap()[0:16, :])
            gb1sb = cpool.tile([64, 2], f32)
            nc.sync.dma_start(gb1sb[:], gb1.ap())
            gb2sb = cpool.tile([64, 2], f32)
            nc.sync.dma_start(gb2sb[:], gb2.ap())
            brsb = cpool.tile([64, 1], f32)
            nc.sync.dma_start(brsb[:], brv.ap())
            ident = cpool.tile([64, 64], f32)
            make_identity(nc, ident[:])

            # ---- P0: camera MLP -> film scale/shift [64,1] ----
            bsel_sb = cpool.tile([64, 4], f32)
            nc.sync.dma_start(bsel_sb[:], bsel.ap())

            fs = cpool.tile([64, 1], f32)
            fsh = cpool.tile([64, 1], f32)
            for wt_dram, out in ((wcs, fs), (wcsh, fsh)):
                pc = psp.tile([64, 4], f32, space="PSUM", tag="ps")
                done = 0
                total = g.cam + 1
                first = True
                while done < total:
                    kk = min(128, total - done)
                    wchunk = stgp.tile([128, 64], f32, tag="wchunk")
                    nc.sync.dma_start(wchunk[0:kk, :],
                                      wt_dram.ap()[done:done + kk, :])
                    cchunk = stgp.tile([128, 4], f32, tag="cchunk")
                    nc.sync.dma_start(cchunk[0:kk, :],
                                      camt.ap()[done:done + kk, :])
                    nc.tensor.matmul(pc[:], wchunk[0:kk, :], cchunk[0:kk, :],
                                     start=first, stop=(done + kk == total))
                    first = False
                    done += kk
                csb = stgp.tile([64, 4], f32, tag="csb")
                nc.vector.tensor_copy(csb[:], pc[:])
                tmp = stgp.tile([64, 4], f32, tag="csb")
                nc.vector.tensor_tensor(out=tmp[:], in0=csb[:], in1=bsel_sb[:],
                                        op=ALU.mult)
                nc.vector.tensor_reduce(out=out[:], in_=tmp[:], axis=AX.X,
                                        op=ALU.add)

            # ---- P-A: conv1 ----
            s1sum = cpool.tile([64, 64], f32)
            s1sq = cpool.tile([64, 64], f32)
            ts0 = g.m // til
            ts1 = (g.m + g.own) // til
            rem1 = (g.m + g.own) % til

            def stage_a(wt_idx, ps, sh):
                if ts0 <= wt_idx < ts1:
                    col = wt_idx - ts0
                    nc.scalar.activation(sh[:], ps[:], AF.Copy,
                                         accum_out=s1sum[:, col:col + 1])
                    scr = stgp.tile([64, til], bf, tag="scr")
                    nc.scalar.activation(scr[:], ps[:], AF.Square,
                                         accum_out=s1sq[:, col:col + 1])
                elif wt_idx == ts1 and rem1:
                    col = ts1 - ts0
                    nc.scalar.activation(sh[:, 0:rem1], ps[:, 0:rem1],
                                         AF.Copy,
                                         accum_out=s1sum[:, col:col + 1])
                    nc.scalar.activation(sh[:, rem1:til], ps[:, rem1:til],
                                         AF.Copy)
                    scr = stgp.tile([64, til], bf, tag="scr")
                    nc.scalar.activation(scr[:, 0:rem1], ps[:, 0:rem1],
                                         AF.Square,
                                         accum_out=s1sq[:, col:col + 1])
                else:
                    nc.scalar.activation(sh[:], ps[:], AF.Copy)

            for j in range(g.nch_a):
                fsrc = bigp.tile([128, g.src_a], u32, tag="bigsrc")
                nc.sync.dma_start(
                    fsrc[:, 0:g.src_a - 1],
                    fp.ap()[:, j * g.a_chunk:j * g.a_chunk + g.src_a - 1])
                nc.vector.memset(fsrc[:, g.src_a - 1:g.src_a], 0)
                idx1c = idxp.tile([128, g.at * 4 * ncol], i16, tag="idx1")
                nc.sync.dma_start(
                    idx1c[:],
                    idx1.ap()[:, j * g.at * 4 * ncol:(j + 1) * g.at * 4 * ncol])

                n_insts = len(G1_TAPS)
                for blk in range((g.at + 1) // 2):
                    bt = min(2, g.at - blk * 2)     # tiles in this block
                    bw = bt * til
                    gobs = []
                    for gi in range(n_insts):
                        nch = G1_TAPS[gi] * 16
                        go = gatp.tile([128, 2 * til], u32, tag="gg")
                        col0 = (blk * 2 * 4 + gi * bt) * ncol
                        nc.gpsimd.ap_gather(
                            out_ap=go[0:nch, 0:bw], in_ap=fsrc[0:nch, :],
                            idxs_ap=idx1c[0:nch, col0:col0 + bt * ncol],
                            channels=nch, num_elems=g.src_a, d=1, num_idxs=bw)
                        gobs.append(go)
                    shw = stgp.tile([64, 2 * til], bf, tag="sh")
                    for tl in range(bt):
                        t = blk * 2 + tl
                        wt_idx = j * g.at + t  # window tile index
                        ps = psp.tile([64, til], f32, space="PSUM", tag="ps")
                        for gi in range(n_insts):
                            nch = G1_TAPS[gi] * 16
                            gob = gobs[gi][:].bitcast(bf).rearrange(
                                "p (n two) -> p n two", two=2)
                            for par in range(2):
                                cb = (gi * 2 + par) * 64
                                nc.tensor.matmul(
                                    ps[:], w1sb[0:nch, cb:cb + 64],
                                    gob[0:nch, tl * til:(tl + 1) * til, par],
                                    start=(gi == 0 and par == 0),
                                    stop=(gi == n_insts - 1 and par == 1))
                        stage_a(wt_idx, ps, shw[:, tl * til:(tl + 1) * til])
                    w0 = (j * g.at + blk * 2) * til
                    nc.sync.dma_start(hh.ap()[:, w0:w0 + bt * til],
                                      shw[:, 0:bt * til])

            # ---- AR1 + BN1 coeffs ----
            ncols1 = (ts1 - ts0) + (1 if rem1 else 0)
            st1 = cpool.tile([64, 2], f32)
            nc.vector.tensor_reduce(out=st1[:, 0:1], in_=s1sum[:, 0:ncols1],
                                    axis=AX.X, op=ALU.add)
            nc.vector.tensor_reduce(out=st1[:, 1:2], in_=s1sq[:, 0:ncols1],
                                    axis=AX.X, op=ALU.add)
            nc.sync.dma_start(ar1i.ap(), st1[:])
            if single:
                nc.sync.dma_start(ar1o.ap(), st1[:])
            else:
                nc.gpsimd.collective_compute(
                    "AllReduce", ALU.add, replica_groups=groups,
                    ins=[ar1i.ap()], outs=[ar1o.ap()])
            ar1sb = cpool.tile([64, 2], f32)
            nc.sync.dma_start(ar1sb[:], ar1o.ap())

            epssb = cpool.tile([64, 1], f32)
            nc.vector.memset(epssb[:], EPS)

            def bn_coeffs(arsb, gbsb, tag):
                mean = stgp.tile([64, 1], f32, tag=tag)
                nc.scalar.mul(mean[:], arsb[:, 0:1], inv_n)
                ex2 = stgp.tile([64, 1], f32, tag=tag)
                nc.scalar.mul(ex2[:], arsb[:, 1:2], inv_n)
                var = stgp.tile([64, 1], f32, tag=tag)
                nc.vector.tensor_tensor(out=var[:], in0=mean[:], in1=mean[:],
                                        op=ALU.mult)
                nc.vector.tensor_tensor(out=var[:], in0=ex2[:], in1=var[:],
                                        op=ALU.subtract)
                sd = stgp.tile([64, 1], f32, tag=tag)
                nc.scalar.activation(sd[:], var[:], AF.Sqrt, bias=epssb[:])
                d = stgp.tile([64, 1], f32, tag=tag)
                nc.vector.reciprocal(d[:], sd[:])
                a = stgp.tile([64, 1], f32, tag=tag)
                nc.vector.tensor_tensor(out=a[:], in0=d[:], in1=gbsb[:, 0:1],
                                        op=ALU.mult)
                b = stgp.tile([64, 1], f32, tag=tag)
                nc.vector.tensor_tensor(out=b[:], in0=mean[:], in1=a[:],
                                        op=ALU.mult)
                nc.vector.tensor_tensor(out=b[:], in0=gbsb[:, 1:2], in1=b[:],
                                        op=ALU.subtract)
                return mean, a, b

            _, a1, b1 = bn_coeffs(ar1sb, gb1sb, "bnc1")
            c1sb = cpool.tile([64, 2], f32)
            nc.vector.tensor_copy(c1sb[:, 0:1], a1[:])
            nc.vector.tensor_copy(c1sb[:, 1:2], b1[:])
            nc.sync.dma_start(c1d.ap(), c1sb[:])
            c1e = cpool.tile([128, 2], f32)
            c1o = cpool.tile([128, 2], f32)
            for r in range(4):
                nc.sync.dma_start(c1e[32 * r:32 * r + 32, :],
                                  c1d.ap()[0:32, :])
                nc.sync.dma_start(c1o[32 * r:32 * r + 32, :],
                                  c1d.ap()[32:64, :])

            # ---- P-C: conv2 ----
            s2sum = cpool.tile([64, 64], f32)
            s2sq = cpool.tile([64, 64], f32)
            SUB = 2816
            for kk in range(g.nch_c):
                h2p = bigp.tile([128, g.src_c], u32, tag="bigsrc")
                h2pb = h2p[:].bitcast(bf).rearrange("p (n two) -> p n two",
                                                    two=2)
                base_w = kk * g.c_chunk
                done = 0
                while done < g.src_c - 1:
                    width = min(SUB, g.src_c - 1 - done)
                    he = ldp.tile([128, SUB], bf, tag="he", bufs=2)
                    ho = ldp.tile([128, SUB], bf, tag="ho", bufs=2)
                    for r in range(4):
                        nc.sync.dma_start(
                            he[32 * r:32 * r + 32, 0:width],
                            hh.ap()[0:32, base_w + done:base_w + done + width])
                        nc.sync.dma_start(
                            ho[32 * r:32 * r + 32, 0:width],
                            hh.ap()[32:64, base_w + done:base_w + done + width])
                    nc.scalar.activation(h2pb[:, done:done + width, 0],
                                         he[:, 0:width], AF.Relu,
                                         bias=c1e[:, 1:2], scale=c1e[:, 0:1])
                    nc.scalar.activation(h2pb[:, done:done + width, 1],
                                         ho[:, 0:width], AF.Relu,
                                         bias=c1o[:, 1:2], scale=c1o[:, 0:1])
                    done += width
                nc.vector.memset(h2p[:, g.src_c - 1:g.src_c], 0)
                idx2c = idxp.tile([128, g.ct * 7 * ncol], i16, tag="idx2")
                nc.sync.dma_start(
                    idx2c[:],
                    idx2.ap()[:, kk * g.ct * 7 * ncol:(kk + 1) * g.ct * 7 * ncol])

                n_insts = len(G2_TAPS)
                for blk in range((g.ct + 1) // 2):
                    bt = min(2, g.ct - blk * 2)
                    bw = bt * til
                    gobs = []
                    for gi in range(n_insts):
                        nch = G2_TAPS[gi] * 32
                        go = gatp.tile([128, 2 * til], u32, tag="gg")
                        col0 = (blk * 2 * 7 + gi * bt) * ncol
                        nc.gpsimd.ap_gather(
                            out_ap=go[0:nch, 0:bw], in_ap=h2p[0:nch, :],
                            idxs_ap=idx2c[0:nch, col0:col0 + bt * ncol],
                            channels=nch, num_elems=g.src_c, d=1, num_idxs=bw)
                        gobs.append(go)
                    sh2w = stgp.tile([64, 2 * til], bf, tag="sh2")
                    for tl in range(bt):
                        t = blk * 2 + tl
                        ot_idx = kk * g.ct + t
                        ps2 = psp.tile([64, til], f32, space="PSUM", tag="ps")
                        for gi in range(n_insts):
                            nch = G2_TAPS[gi] * 32
                            gob = gobs[gi][:].bitcast(bf).rearrange(
                                "p (n two) -> p n two", two=2)
                            for par in range(2):
                                cb = (gi * 2 + par) * 64
                                nc.tensor.matmul(
                                    ps2[:], w2sb[0:nch, cb:cb + 64],
                                    gob[0:nch, tl * til:(tl + 1) * til, par],
                                    start=(gi == 0 and par == 0),
                                    stop=(gi == n_insts - 1 and par == 1))
                        nc.scalar.activation(
                            sh2w[:, tl * til:(tl + 1) * til], ps2[:], AF.Copy,
                            accum_out=s2sum[:, ot_idx:ot_idx + 1])
                        scr2 = stgp.tile([64, til], bf, tag="scr")
                        nc.scalar.activation(
                            scr2[:], ps2[:], AF.Square,
                            accum_out=s2sq[:, ot_idx:ot_idx + 1])
                    o0 = (kk * g.ct + blk * 2) * til
                    nc.sync.dma_start(h2pre.ap()[:, o0:o0 + bt * til],
                                      sh2w[:, 0:bt * til])

            # ---- AR2 + BN2*FiLM coeffs ----
            ncols2 = g.own_p // til
            st2 = cpool.tile([64, 2], f32)
            nc.vector.tensor_reduce(out=st2[:, 0:1], in_=s2sum[:, 0:ncols2],
                                    axis=AX.X, op=ALU.add)
            nc.vector.tensor_reduce(out=st2[:, 1:2], in_=s2sq[:, 0:ncols2],
                                    axis=AX.X, op=ALU.add)
            nc.sync.dma_start(ar2i.ap(), st2[:])
            if single:
                nc.sync.dma_start(ar2o.ap(), st2[:])
            else:
                nc.gpsimd.collective_compute(
                    "AllReduce", ALU.add, replica_groups=groups,
                    ins=[ar2i.ap()], outs=[ar2o.ap()])
            ar2sb = cpool.tile([64, 2], f32)
            nc.sync.dma_start(ar2sb[:], ar2o.ap())
            _, a2r, b2r = bn_coeffs(ar2sb, gb2sb, "bnc2")
            fs1 = cpool.tile([64, 1], f32)
            nc.vector.tensor_scalar(out=fs1[:], in0=fs[:], scalar1=1.0,
                                    scalar2=None, op0=ALU.add)
            a2 = cpool.tile([64, 1], f32)
            nc.vector.tensor_tensor(out=a2[:], in0=a2r[:], in1=fs1[:],
                                    op=ALU.mult)
            b2 = cpool.tile([64, 1], f32)
            nc.vector.tensor_tensor(out=b2[:], in0=b2r[:], in1=fs1[:],
                                    op=ALU.mult)
            nc.vector.tensor_tensor(out=b2[:], in0=b2[:], in1=fsh[:],
                                    op=ALU.add)

            # ---- P-D: epilogue (1024-wide blocks) ----
            f_off = 2 * g.m  # own col o <-> fp col o + 2m
            n_til_d = g.own_p // til
            for blk in range((n_til_d + 1) // 2):
                bt = min(2, n_til_d - blk * 2)
                bw = bt * til
                o0 = blk * 2 * til
                h2t = ldp.tile([64, 2 * til], bf, tag="h2t")
                nc.sync.dma_start(h2t[:, 0:bw], h2pre.ap()[:, o0:o0 + bw])
                rhsid = ldp.tile([16, 2 * til], u32, tag="rhsid")
                nc.sync.dma_start(rhsid[:, 0:bw],
                                  fp.ap()[0:16, f_off + o0:f_off + o0 + bw])
                rb = rhsid[:].bitcast(bf).rearrange("p (n two) -> p n two",
                                                    two=2)
                t1 = stgp.tile([64, 2 * til], f32, tag="t1")
                nc.scalar.activation(t1[:, 0:bw], h2t[:, 0:bw], AF.Relu,
                                     bias=b2[:], scale=a2[:])
                t2 = stgp.tile([64, 2 * til], f32, tag="t2")
                for tl in range(bt):
                    s0 = tl * til
                    psid = psp.tile([64, til], f32, space="PSUM", tag="ps")
                    nc.tensor.matmul(psid[:], wrsb[0:16, 0:64],
                                     rb[0:16, s0:s0 + til, 0],
                                     start=True, stop=False)
                    nc.tensor.matmul(psid[:], wrsb[0:16, 64:128],
                                     rb[0:16, s0:s0 + til, 1],
                                     start=False, stop=True)
                    nc.vector.scalar_tensor_tensor(
                        out=t2[:, s0:s0 + til], in0=psid[:], scalar=brsb[:],
                        in1=t1[:, s0:s0 + til], op0=ALU.add, op1=ALU.add)
                ost = stgp.tile([128, 8 * 64], f32, tag="ost")
                pst = ptp.tile([128, 8 * 64], f32, space="PSUM", tag="pst")
                for q in range(bw // 128):
                    nc.tensor.transpose(pst[:, q * 64:(q + 1) * 64],
                                        t2[:, q * 128:(q + 1) * 128],
                                        ident[:])
                nc.vector.tensor_copy(ost[:, 0:bw // 2], pst[:, 0:bw // 2])
                dst = bass.AP(tensor=y, offset=o0 * 64,
                              ap=[[64, 128], [128 * 64, bw // 128], [1, 64]])
                nc.sync.dma_start(dst, ost[:, 0:bw // 2])

    nc.compile()
    return nc


# ---------------------------------------------------------------------------
# Host-side preparation
# ---------------------------------------------------------------------------

def _pack_pairs(x):
    """[n, C] f32 -> [C//2, n] uint32 of bf16 (even|odd<<16) pairs."""
    xb = x.astype(BF16)
    lo = xb[:, 0::2].view(np.uint16).astype(np.uint32)
    hi = xb[:, 1::2].view(np.uint16).astype(np.uint32)
    return np.ascontiguousarray((lo | (hi << 16)).T)


def _wrap1(iv, g):
    """conv1 idx block for one chunk: iv [a_chunk, 27] -> [128, at*4*ncol].

    Column order matches the device loop: per 2-tile block, per gather inst
    (gi), the whole block's bw=bt*til indices wrapped [16-lane, bw/16-col]
    per tap group.
    """
    til = g.til
    ncol = til // 16
    cols = []
    for blk in range((g.at + 1) // 2):
        bt = min(2, g.at - blk * 2)
        bw = bt * til
        B = iv[blk * 2 * til: blk * 2 * til + bw]          # [bw, 27]
        A = B.reshape(bw // 16, 16, g.k).transpose(2, 1, 0)  # [27,16,bw/16]
        Bp = np.zeros((32, 16, bw // 16), np.int16)
        Bp[:g.k] = A
        for gi in range(4):
            cols.append(Bp[gi * 8:(gi + 1) * 8].reshape(128, bw // 16))
    return np.concatenate(cols, 1)                        # [128, at*4*ncol]


def _wrap2(iv, g):
    """conv2 idx block for one chunk: iv [c_chunk, 27] -> [128, ct*7*ncol]."""
    til = g.til
    ncol = til // 16
    cols = []
    for blk in range((g.ct + 1) // 2):
        bt = min(2, g.ct - blk * 2)
        bw = bt * til
        B = iv[blk * 2 * til: blk * 2 * til + bw]
        A = B.reshape(bw // 16, 16, g.k).transpose(2, 1, 0)  # [27,16,bw/16]
        Bp = np.zeros((28, 16, bw // 16), np.int16)
        Bp[:g.k] = A
        C = Bp.reshape(7, 4, 1, 16, bw // 16)
        D = np.broadcast_to(C, (7, 4, 2, 16, bw // 16))
        for gi in range(7):
            cols.append(np.ascontiguousarray(D[gi]).reshape(128, bw // 16))
    return np.concatenate(cols, 1)                        # [128, ct*7*ncol]


def prepare_inputs(g: Geo, feats, camera_cond, W1, g1, be1, W2, g2, be2,
                   Wc, bc, Wr, br, nbr, batch_idx):
    n = g.n
    til = g.til

    pos = np.arange(n, dtype=np.int64)
    dmax = 0
    for t in range(g.k):
        col = nbr[:, t].astype(np.int64)
        v = col >= 0
        if v.any():
            dmax = max(dmax, int(np.abs(col[v] - pos[v]).max()))
    assert dmax <= g.m, f"neighbor margin {dmax} exceeds {g.m}"

    fpg = _pack_pairs(feats)                      # [16, n]

    # weights
    W1b = np.asarray(W1, F32)
    wpk1 = np.zeros((128, 8 * 64), BF16)
    for gi in range(4):
        for q in range(G1_TAPS[gi]):
            t = gi * 8 + q
            for par in range(2):
                cb = (gi * 2 + par) * 64
                # rows q*16+j = W1[t, 2j+par, PERM1[m]]
                wpk1[q * 16:(q + 1) * 16, cb:cb + 64] = (
                    W1b[t, par::2, :][:, PERM1].astype(BF16))
    W2b = np.asarray(W2, F32)
    wpk2 = np.zeros((128, 14 * 64), BF16)
    for gi in range(7):
        for u in range(G2_TAPS[gi]):
            t = gi * 4 + u
            for par in range(2):
                cb = (gi * 2 + par) * 64
                wpk2[u * 32:(u + 1) * 32, cb:cb + 64] = (
                    W2b[t, par::2, :].astype(BF16))
    Wrb = np.asarray(Wr, F32)
    wrt = np.zeros((17, 128), BF16)
    wrt[0:16, 0:64] = Wrb[0::2, :].astype(BF16)
    wrt[0:16, 64:128] = Wrb[1::2, :].astype(BF16)
    brv = np.asarray(br, F32).reshape(64, 1)

    Wcn = np.asarray(Wc, F32)
    bcn = np.asarray(bc, F32)
    wcs = np.concatenate([Wcn[:, 0:64], bcn[None, 0:64]], 0).astype(F32)
    wcsh = np.concatenate([Wcn[:, 64:128], bcn[None, 64:128]], 0).astype(F32)
    camt = np.concatenate([np.asarray(camera_cond, F32).T,
                           np.ones((1, 4), F32)], 0)

    gb1 = np.stack([np.asarray(g1, F32)[PERM1],
                    np.asarray(be1, F32)[PERM1]], 1)
    gb2 = np.stack([np.asarray(g2, F32), np.asarray(be2, F32)], 1)

    nbr64 = nbr.astype(np.int64)
    valid_all = nbr64 >= 0

    in_maps = []
    for c in range(g.n_cores):
        own0 = c * g.own
        win0 = own0 - g.m
        f0 = own0 - 2 * g.m

        # feats window, replicated 8x along partitions for the 8-tap gathers
        fpc1 = np.zeros((16, g.fwinp), np.uint32)
        lo = max(f0, 0)
        hi = min(f0 + g.fwinp, n)
        if hi > lo:
            fpc1[:, lo - f0:hi - f0] = fpg[:, lo:hi]
        fpc = np.tile(fpc1, (8, 1))

        # conv1 indices
        Gw = win0 + np.arange(g.win)
        inb = (Gw >= 0) & (Gw < n)
        Gc = np.clip(Gw, 0, n - 1)
        src = np.where((inb[:, None]) & valid_all[Gc], nbr64[Gc], -1)
        blocks = []
        for j in range(g.nch_a):
            rows = slice(j * g.a_chunk, (j + 1) * g.a_chunk)
            sr = src[rows]
            loc = sr - (f0 + j * g.a_chunk)
            iv = np.where(sr >= 0, loc, g.src_a - 1)
            assert iv.min() >= 0 and iv.max() <= g.src_a - 1
            blocks.append(_wrap1(iv.astype(np.int16), g))
        idx1 = np.concatenate(blocks, 1)

        # conv2 indices
        Go = own0 + np.arange(g.own_p)
        inb2 = Go < own0 + g.own
        Gc2 = np.clip(Go, 0, n - 1)
        src2 = np.where((inb2[:, None]) & valid_all[Gc2], nbr64[Gc2], -1)
        blocks2 = []
        for kk in range(g.nch_c):
            rows = slice(kk * g.c_chunk, (kk + 1) * g.c_chunk)
            sr = src2[rows]
            loc = sr - (win0 + kk * g.c_chunk)
            iv = np.where(sr >= 0, loc, g.src_c - 1)
            assert iv.min() >= 0 and iv.max() <= g.src_c - 1
            blocks2.append(_wrap2(iv.astype(np.int16), g))
        idx2 = np.concatenate(blocks2, 1)

        b = int(batch_idx[own0])
        bsel = np.zeros((64, 4), F32)
        bsel[:, b] = 1.0

        in_maps.append({
            "fp": fpc, "idx1": idx1, "idx2": idx2,
            "wpk1": wpk1, "wpk2": wpk2, "wrt": wrt,
            "wcs": wcs, "wcsh": wcsh, "camt": camt, "bsel": bsel,
            "gb1": gb1, "gb2": gb2, "brv": brv,
        })
    return in_maps


# ---------------------------------------------------------------------------
# Entry point
# ---------------------------------------------------------------------------

_NC_CACHE = {}


def _get_module(g: Geo):
    if g not in _NC_CACHE:
        _NC_CACHE[g] = build_module(g)
    return _NC_CACHE[g]


def kernel(**inputs) -> np.ndarray:
    g = GEO_FULL
    nc = _get_module(g)
    args = {k: np.asarray(v) for k, v in inputs.items()}
    in_maps = prepare_inputs(
        g, args["feats"], args["camera_cond"], args["W1"], args["g1"],
        args["be1"], args["W2"], args["g2"], args["be2"], args["Wc"],
        args["bc"], args["Wr"], args["br"], args["nbr"], args["batch_idx"])
    res = run_bass_kernel_spmd(nc, in_maps, core_ids=list(range(g.n_cores)))
    out = np.concatenate(
        [res.results[c]["y"][:g.own] for c in range(g.n_cores)], 0)
    return out.astype(np.float32)



# revision 2
# speedup vs baseline: 1.9883x; 1.9883x over previous
"""Trainium2 Bass kernel for nn_CameraAwareSparseBlock (sparse submanifold 3x3x3
conv x2 + BN + FiLM + residual), data-parallel over 8 NeuronCores.

Strategy (per core, SPMD):
  - Each core owns N/8 consecutive voxels (voxels are sorted by batch-major key,
    so neighbor indices stay within +-margin positions of each output row).
  - Features are kept TRANSPOSED and bf16 channel-PAIR-packed in uint32 words:
    source layout [pair_channels, voxels]. GPSIMD ap_gather gathers arbitrary
    voxel columns for 8 taps (conv1, 16 pair-rows/tap) or 4 taps (conv2,
    32 pair-rows/tap) per instruction into a [128, 512] tile that feeds the
    TensorEngine directly: two matmuls per gather (even/odd bf16 halves via
    stride-2 access patterns) contract K=128 = taps x pair-channels.
  - conv1 (P-A) streams windowed source chunks, accumulates BN stats from f32
    PSUM, stages pre-BN outputs (even/odd channel split) to HBM.
  - AllReduce (sum, sumsq) across 8 cores -> BN1 affine coeffs.
  - conv2 (P-C) reloads staged outputs with 4x partition replication, applies
    BN1 affine + ReLU while pair-packing, gathers + matmuls, stages pre-BN2
    outputs, accumulates BN2 stats.
  - AllReduce -> BN2 + FiLM coeffs (camera MLP computed on-device).
  - Epilogue (P-D): affine+ReLU, adds the 1x1 residual (matmul from the packed
    feature source), transposes via PE, writes [N/8, 64] f32 rows.
"""

import numpy as np
import ml_dtypes
from contextlib import ExitStack
from dataclasses import dataclass

import concourse.bass as bass
import concourse.tile as tile
from concourse import bacc, mybir
from concourse.bass_utils import run_bass_kernel_spmd
from concourse.masks import make_identity

BF16 = ml_dtypes.bfloat16
F32 = np.float32
AF = mybir.ActivationFunctionType
ALU = mybir.AluOpType
AX = mybir.AxisListType

EPS = 1e-5


@dataclass(frozen=True)
class Geo:
    n: int          # total voxels
    n_cores: int    # 8
    til: int        # 512
    m: int          # neighbor position margin (multiple of til)
    at: int         # tiles per P-A chunk
    nch_a: int      # P-A chunks
    ct: int         # tiles per P-C chunk
    nch_c: int      # P-C chunks
    c1: int = 32
    c2: int = 64
    k: int = 27
    cam: int = 256

    @property
    def own(self):
        return self.n // self.n_cores

    @property
    def a_chunk(self):
        return self.at * self.til

    @property
    def c_chunk(self):
        return self.ct * self.til

    @property
    def win(self):
        return self.nch_a * self.a_chunk

    @property
    def own_p(self):
        return self.nch_c * self.c_chunk

    @property
    def src_a(self):
        return self.a_chunk + 2 * self.m + 1

    @property
    def src_c(self):
        return self.c_chunk + 2 * self.m + 1

    @property
    def fwinp(self):
        return (self.nch_a - 1) * self.a_chunk + self.src_a

    @property
    def winp(self):
        return max(self.win, (self.nch_c - 1) * self.c_chunk + self.src_c)

    def check(self):
        assert self.m % self.til == 0
        assert self.win >= self.own + 2 * self.m
        assert self.own_p >= self.own and self.own_p - self.own < self.til
        assert self.src_a <= 32768 and self.src_c <= 32768
        assert self.n % self.n_cores == 0


GEO_FULL = Geo(n=200000, n_cores=8, til=512, m=1024, at=8, nch_a=7, ct=7, nch_c=7)

PERM1 = np.array([2 * i for i in range(32)] + [2 * i + 1 for i in range(32)])


def _wb_layout():
    off = 0
    L = {}

    def add(name, rows, cols_u32):
        nonlocal off
        L[name] = (off, rows, cols_u32)
        off += rows * cols_u32

    add("w1", 128, 8 * 64 // 2)
    add("w2", 128, 14 * 64 // 2)
    add("wr", 16, 128 // 2)
    add("wcsa", 128, 64)
    add("wcsb", 128, 64)
    add("wcsc", 1, 64)
    add("wcsha", 128, 64)
    add("wcshb", 128, 64)
    add("wcshc", 1, 64)
    add("camta", 128, 4)
    add("camtb", 128, 4)
    add("camtc", 1, 4)
    add("bsel", 64, 4)
    add("gb1", 64, 2)
    add("gb2", 64, 2)
    add("brv", 64, 1)
    return L, off


WB_L, WB_W = _wb_layout()

# conv1: 4 gather insts per tile covering taps [0:8),[8:16),[16:24),[24:27)
G1_TAPS = [8, 8, 8, 3]
# conv2: 7 gather insts covering 4,4,4,4,4,4,3 taps
G2_TAPS = [4, 4, 4, 4, 4, 4, 3]


# ---------------------------------------------------------------------------
# Device program
# ---------------------------------------------------------------------------

def build_module(g: Geo, single: bool = False):
    """single=True builds a 1-core variant with AllReduce replaced by a copy,
    for TimelineSim cost-model analysis (collectives unsupported there)."""
    g.check()
    nc = bacc.Bacc("TRN2", target_bir_lowering=False, debug=False,
                   num_devices=(1 if single else g.n_cores))
    u32, i16, bf, f32 = (mybir.dt.uint32, mybir.dt.int16,
                         mybir.dt.bfloat16, mybir.dt.float32)
    til = g.til
    ncol = til // 16

    # ---- I/O tensors (per core) ----
    # fp: host-replicated pair-packed features [128 = 8 reps x 16 pairs, fwinp]
    fp = nc.dram_tensor("fp", [16, g.fwinp], u32, kind="ExternalInput")
    idx1 = nc.dram_tensor("idx1", [128, g.nch_a * g.at * 4 * ncol], i16,
                          kind="ExternalInput")
    idx2 = nc.dram_tensor("idx2", [128, g.nch_c * g.ct * 7 * ncol], i16,
                          kind="ExternalInput")
    wb = nc.dram_tensor("wb", [1, WB_W], u32, kind="ExternalInput")
    y = nc.dram_tensor("y", [64, g.own], mybir.dt.float16,
                       kind="ExternalOutput")

    def wap(name, dt):
        off, rows, cols = WB_L[name]
        return wb.ap()[0:1, off:off + rows * cols].bitcast(dt)

    # ---- internal DRAM ----
    # conv1 pre-BN output, rows 0:32 = even channels (PERM1), 32:64 = odd
    hh = nc.dram_tensor("hh", [64, g.winp], bf, kind="Internal")
    h2pre = nc.dram_tensor("h2pre", [64, g.own_p], bf, kind="Internal")
    ar1i = nc.dram_tensor("ar1i", [64, 2], f32, kind="Internal")
    ar1o = nc.dram_tensor("ar1o", [64, 2], f32, kind="Internal")
    ar2i = nc.dram_tensor("ar2i", [64, 2], f32, kind="Internal")
    ar2o = nc.dram_tensor("ar2o", [64, 2], f32, kind="Internal")
    c1d = nc.dram_tensor("c1d", [64, 2], f32, kind="Internal")

    groups = [list(range(g.n_cores))]
    inv_n = 1.0 / float(g.n)

    with tile.TileContext(nc) as tc:
        with ExitStack() as ctx:
            cpool = ctx.enter_context(tc.tile_pool(name="const", bufs=1))
            bigp = ctx.enter_context(tc.tile_pool(name="bigsrc", bufs=2))
            gatp = ctx.enter_context(tc.tile_pool(name="gat", bufs=9))
            idxp = ctx.enter_context(tc.tile_pool(name="idx", bufs=2))
            stgp = ctx.enter_context(tc.tile_pool(name="stg", bufs=3))
            ldp = ctx.enter_context(tc.tile_pool(name="ld", bufs=3))
            psp = ctx.enter_context(tc.tile_pool(name="ps", bufs=6, space="PSUM"))

            # ---- constants ----
            w1sb = cpool.tile([128, 8 * 64], bf)
            nc.sync.dma_start(w1sb[:], wap("w1", bf))
            w2sb = cpool.tile([128, 14 * 64], bf)
            nc.sync.dma_start(w2sb[:], wap("w2", bf))
            wrsb = cpool.tile([16, 128], bf)
            nc.sync.dma_start(wrsb[:], wap("wr", bf))
            gb1sb = cpool.tile([64, 2], f32)
            nc.sync.dma_start(gb1sb[:], wap("gb1", f32))
            gb2sb = cpool.tile([64, 2], f32)
            nc.sync.dma_start(gb2sb[:], wap("gb2", f32))
            brsb = cpool.tile([64, 1], f32)
            nc.sync.dma_start(brsb[:], wap("brv", f32))
            # ---- P0: camera MLP -> film scale/shift [64,1] ----
            bsel_sb = cpool.tile([64, 4], f32)
            nc.sync.dma_start(bsel_sb[:], wap("bsel", f32))

            fs = cpool.tile([64, 1], f32)
            fsh = cpool.tile([64, 1], f32)
            for wname, out in (("wcs", fs), ("wcsh", fsh)):
                pc = psp.tile([64, 4], f32, space="PSUM", tag="ps")
                for ci, (wn, cn, kk) in enumerate(
                        [(wname + "a", "camta", 128),
                         (wname + "b", "camtb", 128),
                         (wname + "c", "camtc", 1)]):
                    wchunk = stgp.tile([128, 64], f32, tag="wchunk")
                    nc.sync.dma_start(wchunk[0:kk, :], wap(wn, f32))
                    cchunk = stgp.tile([128, 4], f32, tag="cchunk")
                    nc.sync.dma_start(cchunk[0:kk, :], wap(cn, f32))
                    nc.tensor.matmul(pc[:], wchunk[0:kk, :], cchunk[0:kk, :],
                                     start=(ci == 0), stop=(ci == 2))
                csb = stgp.tile([64, 4], f32, tag="csb")
                nc.vector.tensor_copy(csb[:], pc[:])
                tmp = stgp.tile([64, 4], f32, tag="csb")
                nc.vector.tensor_tensor(out=tmp[:], in0=csb[:], in1=bsel_sb[:],
                                        op=ALU.mult)
                nc.vector.tensor_reduce(out=out[:], in_=tmp[:], axis=AX.X,
                                        op=ALU.add)

            # ---- P-A: conv1 ----
            s1sum = cpool.tile([64, 64], f32)
            s1sq = cpool.tile([64, 64], f32)
            ts0 = g.m // til
            ts1 = (g.m + g.own) // til
            rem1 = (g.m + g.own) % til

            def stage_a(wt_idx, ps, sh):
                if ts0 <= wt_idx < ts1:
                    col = wt_idx - ts0
                    nc.scalar.activation(sh[:], ps[:], AF.Copy,
                                         accum_out=s1sum[:, col:col + 1])
                    scr = stgp.tile([64, til], bf, tag="scr")
                    nc.scalar.activation(scr[:], ps[:], AF.Square,
                                         accum_out=s1sq[:, col:col + 1])
                elif wt_idx == ts1 and rem1:
                    col = ts1 - ts0
                    nc.scalar.activation(sh[:, 0:rem1], ps[:, 0:rem1],
                                         AF.Copy,
                                         accum_out=s1sum[:, col:col + 1])
                    nc.scalar.activation(sh[:, rem1:til], ps[:, rem1:til],
                                         AF.Copy)
                    scr = stgp.tile([64, til], bf, tag="scr")
                    nc.scalar.activation(scr[:, 0:rem1], ps[:, 0:rem1],
                                         AF.Square,
                                         accum_out=s1sq[:, col:col + 1])
                else:
                    nc.scalar.activation(sh[:], ps[:], AF.Copy)

            for j in range(g.nch_a):
                fsrc = bigp.tile([128, g.src_a], u32, tag="bigsrc")
                for r in range(8):
                    nc.sync.dma_start(
                        fsrc[r * 16:r * 16 + 16, 0:g.src_a - 1],
                        fp.ap()[0:16,
                                j * g.a_chunk:j * g.a_chunk + g.src_a - 1])
                nc.vector.memset(fsrc[:, g.src_a - 1:g.src_a], 0)
                idx1c = idxp.tile([128, g.at * 4 * ncol], i16, tag="idx1")
                nc.sync.dma_start(
                    idx1c[:],
                    idx1.ap()[:, j * g.at * 4 * ncol:(j + 1) * g.at * 4 * ncol])

                n_insts = len(G1_TAPS)
                for blk in range((g.at + 1) // 2):
                    bt = min(2, g.at - blk * 2)     # tiles in this block
                    bw = bt * til
                    gobs = []
                    for gi in range(n_insts):
                        nch = G1_TAPS[gi] * 16
                        go = gatp.tile([128, 2 * til], u32, tag="gg")
                        col0 = (blk * 2 * 4 + gi * bt) * ncol
                        nc.gpsimd.ap_gather(
                            out_ap=go[0:nch, 0:bw], in_ap=fsrc[0:nch, :],
                            idxs_ap=idx1c[0:nch, col0:col0 + bt * ncol],
                            channels=nch, num_elems=g.src_a, d=1, num_idxs=bw)
                        gobs.append(go)
                    shw = stgp.tile([64, 2 * til], bf, tag="sh")
                    for tl in range(bt):
                        t = blk * 2 + tl
                        wt_idx = j * g.at + t  # window tile index
                        ps = psp.tile([64, til], f32, space="PSUM", tag="ps")
                        for gi in range(n_insts):
                            nch = G1_TAPS[gi] * 16
                            gob = gobs[gi][:].bitcast(bf).rearrange(
                                "p (n two) -> p n two", two=2)
                            for par in range(2):
                                cb = (gi * 2 + par) * 64
                                nc.tensor.matmul(
                                    ps[:], w1sb[0:nch, cb:cb + 64],
                                    gob[0:nch, tl * til:(tl + 1) * til, par],
                                    start=(gi == 0 and par == 0),
                                    stop=(gi == n_insts - 1 and par == 1))
                        stage_a(wt_idx, ps, shw[:, tl * til:(tl + 1) * til])
                    w0 = (j * g.at + blk * 2) * til
                    nc.sync.dma_start(hh.ap()[:, w0:w0 + bt * til],
                                      shw[:, 0:bt * til])

            # ---- AR1 + BN1 coeffs ----
            ncols1 = (ts1 - ts0) + (1 if rem1 else 0)
            st1 = cpool.tile([64, 2], f32)
            nc.vector.tensor_reduce(out=st1[:, 0:1], in_=s1sum[:, 0:ncols1],
                                    axis=AX.X, op=ALU.add)
            nc.vector.tensor_reduce(out=st1[:, 1:2], in_=s1sq[:, 0:ncols1],
                                    axis=AX.X, op=ALU.add)
            nc.sync.dma_start(ar1i.ap(), st1[:])
            if single:
                nc.sync.dma_start(ar1o.ap(), st1[:])
            else:
                nc.gpsimd.collective_compute(
                    "AllReduce", ALU.add, replica_groups=groups,
                    ins=[ar1i.ap()], outs=[ar1o.ap()])
            ar1sb = cpool.tile([64, 2], f32)
            nc.sync.dma_start(ar1sb[:], ar1o.ap())

            epssb = cpool.tile([64, 1], f32)
            nc.vector.memset(epssb[:], EPS)

            def bn_coeffs(arsb, gbsb, tag):
                mean = stgp.tile([64, 1], f32, tag=tag)
                nc.scalar.mul(mean[:], arsb[:, 0:1], inv_n)
                ex2 = stgp.tile([64, 1], f32, tag=tag)
                nc.scalar.mul(ex2[:], arsb[:, 1:2], inv_n)
                var = stgp.tile([64, 1], f32, tag=tag)
                nc.vector.tensor_tensor(out=var[:], in0=mean[:], in1=mean[:],
                                        op=ALU.mult)
                nc.vector.tensor_tensor(out=var[:], in0=ex2[:], in1=var[:],
                                        op=ALU.subtract)
                sd = stgp.tile([64, 1], f32, tag=tag)
                nc.scalar.activation(sd[:], var[:], AF.Sqrt, bias=epssb[:])
                d = stgp.tile([64, 1], f32, tag=tag)
                nc.vector.reciprocal(d[:], sd[:])
                a = stgp.tile([64, 1], f32, tag=tag)
                nc.vector.tensor_tensor(out=a[:], in0=d[:], in1=gbsb[:, 0:1],
                                        op=ALU.mult)
                b = stgp.tile([64, 1], f32, tag=tag)
                nc.vector.tensor_tensor(out=b[:], in0=mean[:], in1=a[:],
                                        op=ALU.mult)
                nc.vector.tensor_tensor(out=b[:], in0=gbsb[:, 1:2], in1=b[:],
                                        op=ALU.subtract)
                return mean, a, b

            _, a1, b1 = bn_coeffs(ar1sb, gb1sb, "bnc1")
            c1sb = cpool.tile([64, 2], f32)
            nc.vector.tensor_copy(c1sb[:, 0:1], a1[:])
            nc.vector.tensor_copy(c1sb[:, 1:2], b1[:])
            nc.sync.dma_start(c1d.ap(), c1sb[:])
            c1e = cpool.tile([128, 2], f32)
            c1o = cpool.tile([128, 2], f32)
            for r in range(4):
                nc.sync.dma_start(c1e[32 * r:32 * r + 32, :],
                                  c1d.ap()[0:32, :])
                nc.sync.dma_start(c1o[32 * r:32 * r + 32, :],
                                  c1d.ap()[32:64, :])

            # ---- P-C: conv2 ----
            s2sum = cpool.tile([64, 64], f32)
            s2sq = cpool.tile([64, 64], f32)
            SUB = 2816
            for kk in range(g.nch_c):
                h2p = bigp.tile([128, g.src_c], u32, tag="bigsrc")
                h2pb = h2p[:].bitcast(bf).rearrange("p (n two) -> p n two",
                                                    two=2)
                base_w = kk * g.c_chunk
                done = 0
                while done < g.src_c - 1:
                    width = min(SUB, g.src_c - 1 - done)
                    he = ldp.tile([128, SUB], bf, tag="he", bufs=2)
                    ho = ldp.tile([128, SUB], bf, tag="ho", bufs=2)
                    for r in range(4):
                        nc.sync.dma_start(
                            he[32 * r:32 * r + 32, 0:width],
                            hh.ap()[0:32, base_w + done:base_w + done + width])
                        nc.sync.dma_start(
                            ho[32 * r:32 * r + 32, 0:width],
                            hh.ap()[32:64, base_w + done:base_w + done + width])
                    nc.scalar.activation(h2pb[:, done:done + width, 0],
                                         he[:, 0:width], AF.Relu,
                                         bias=c1e[:, 1:2], scale=c1e[:, 0:1])
                    nc.scalar.activation(h2pb[:, done:done + width, 1],
                                         ho[:, 0:width], AF.Relu,
                                         bias=c1o[:, 1:2], scale=c1o[:, 0:1])
                    done += width
                nc.vector.memset(h2p[:, g.src_c - 1:g.src_c], 0)
                idx2c = idxp.tile([128, g.ct * 7 * ncol], i16, tag="idx2")
                nc.sync.dma_start(
                    idx2c[:],
                    idx2.ap()[:, kk * g.ct * 7 * ncol:(kk + 1) * g.ct * 7 * ncol])

                n_insts = len(G2_TAPS)
                for blk in range((g.ct + 1) // 2):
                    bt = min(2, g.ct - blk * 2)
                    bw = bt * til
                    gobs = []
                    for gi in range(n_insts):
                        nch = G2_TAPS[gi] * 32
                        go = gatp.tile([128, 2 * til], u32, tag="gg")
                        col0 = (blk * 2 * 7 + gi * bt) * ncol
                        nc.gpsimd.ap_gather(
                            out_ap=go[0:nch, 0:bw], in_ap=h2p[0:nch, :],
                            idxs_ap=idx2c[0:nch, col0:col0 + bt * ncol],
                            channels=nch, num_elems=g.src_c, d=1, num_idxs=bw)
                        gobs.append(go)
                    sh2w = stgp.tile([64, 2 * til], bf, tag="sh2")
                    for tl in range(bt):
                        t = blk * 2 + tl
                        ot_idx = kk * g.ct + t
                        ps2 = psp.tile([64, til], f32, space="PSUM", tag="ps")
                        for gi in range(n_insts):
                            nch = G2_TAPS[gi] * 32
                            gob = gobs[gi][:].bitcast(bf).rearrange(
                                "p (n two) -> p n two", two=2)
                            for par in range(2):
                                cb = (gi * 2 + par) * 64
                                nc.tensor.matmul(
                                    ps2[:], w2sb[0:nch, cb:cb + 64],
                                    gob[0:nch, tl * til:(tl + 1) * til, par],
                                    start=(gi == 0 and par == 0),
                                    stop=(gi == n_insts - 1 and par == 1))
                        nc.scalar.activation(
                            sh2w[:, tl * til:(tl + 1) * til], ps2[:], AF.Copy,
                            accum_out=s2sum[:, ot_idx:ot_idx + 1])
                        scr2 = stgp.tile([64, til], bf, tag="scr")
                        nc.scalar.activation(
                            scr2[:], ps2[:], AF.Square,
                            accum_out=s2sq[:, ot_idx:ot_idx + 1])
                    o0 = (kk * g.ct + blk * 2) * til
                    nc.sync.dma_start(h2pre.ap()[:, o0:o0 + bt * til],
                                      sh2w[:, 0:bt * til])

            # ---- AR2 + BN2*FiLM coeffs ----
            ncols2 = g.own_p // til
            st2 = cpool.tile([64, 2], f32)
            nc.vector.tensor_reduce(out=st2[:, 0:1], in_=s2sum[:, 0:ncols2],
                                    axis=AX.X, op=ALU.add)
            nc.vector.tensor_reduce(out=st2[:, 1:2], in_=s2sq[:, 0:ncols2],
                                    axis=AX.X, op=ALU.add)
            nc.sync.dma_start(ar2i.ap(), st2[:])
            if single:
                nc.sync.dma_start(ar2o.ap(), st2[:])
            else:
                nc.gpsimd.collective_compute(
                    "AllReduce", ALU.add, replica_groups=groups,
                    ins=[ar2i.ap()], outs=[ar2o.ap()])
            ar2sb = cpool.tile([64, 2], f32)
            nc.sync.dma_start(ar2sb[:], ar2o.ap())
            _, a2r, b2r = bn_coeffs(ar2sb, gb2sb, "bnc2")
            fs1 = cpool.tile([64, 1], f32)
            nc.vector.tensor_scalar(out=fs1[:], in0=fs[:], scalar1=1.0,
                                    scalar2=None, op0=ALU.add)
            a2 = cpool.tile([64, 1], f32)
            nc.vector.tensor_tensor(out=a2[:], in0=a2r[:], in1=fs1[:],
                                    op=ALU.mult)
            b2 = cpool.tile([64, 1], f32)
            nc.vector.tensor_tensor(out=b2[:], in0=b2r[:], in1=fs1[:],
                                    op=ALU.mult)
            nc.vector.tensor_tensor(out=b2[:], in0=b2[:], in1=fsh[:],
                                    op=ALU.add)

            # ---- P-D: epilogue (1024-wide blocks) ----
            f_off = 2 * g.m  # own col o <-> fp col o + 2m
            n_til_d = g.own_p // til
            for blk in range((n_til_d + 1) // 2):
                bt = min(2, n_til_d - blk * 2)
                bw = bt * til
                o0 = blk * 2 * til
                h2t = ldp.tile([64, 2 * til], bf, tag="h2t")
                nc.sync.dma_start(h2t[:, 0:bw], h2pre.ap()[:, o0:o0 + bw])
                rhsid = ldp.tile([16, 2 * til], u32, tag="rhsid")
                nc.sync.dma_start(rhsid[:, 0:bw],
                                  fp.ap()[0:16, f_off + o0:f_off + o0 + bw])
                rb = rhsid[:].bitcast(bf).rearrange("p (n two) -> p n two",
                                                    two=2)
                t1 = stgp.tile([64, 2 * til], f32, tag="t1")
                nc.scalar.activation(t1[:, 0:bw], h2t[:, 0:bw], AF.Relu,
                                     bias=b2[:], scale=a2[:])
                t2 = stgp.tile([64, 2 * til], mybir.dt.float16, tag="t2")
                ow = min(bw, g.own - o0)
                for tl in range(bt):
                    s0 = tl * til
                    if s0 >= ow:
                        break
                    psid = psp.tile([64, til], f32, space="PSUM", tag="ps")
                    nc.tensor.matmul(psid[:], wrsb[0:16, 0:64],
                                     rb[0:16, s0:s0 + til, 0],
                                     start=True, stop=False)
                    nc.tensor.matmul(psid[:], wrsb[0:16, 64:128],
                                     rb[0:16, s0:s0 + til, 1],
                                     start=False, stop=True)
                    nc.vector.scalar_tensor_tensor(
                        out=t2[:, s0:s0 + til], in0=psid[:], scalar=brsb[:],
                        in1=t1[:, s0:s0 + til], op0=ALU.add, op1=ALU.add)
                nc.sync.dma_start(y.ap()[:, o0:o0 + ow], t2[:, 0:ow])

    nc.compile()
    return nc


# ---------------------------------------------------------------------------
# Host-side preparation
# ---------------------------------------------------------------------------

def _pack_pairs(x):
    """[n, C] f32 -> [C//2, n] uint32 of bf16 (even|odd<<16) pairs."""
    xb = x.astype(BF16)
    lo = xb[:, 0::2].view(np.uint16).astype(np.uint32)
    hi = xb[:, 1::2].view(np.uint16).astype(np.uint32)
    return np.ascontiguousarray((lo | (hi << 16)).T)


def _wrap1(iv, g):
    """conv1 idx block for one chunk: iv [a_chunk, 27] -> [128, at*4*ncol].

    Column order matches the device loop: per 2-tile block, per gather inst
    (gi), the whole block's bw=bt*til indices wrapped [16-lane, bw/16-col]
    per tap group.
    """
    til = g.til
    ncol = til // 16
    cols = []
    for blk in range((g.at + 1) // 2):
        bt = min(2, g.at - blk * 2)
        bw = bt * til
        B = iv[blk * 2 * til: blk * 2 * til + bw]          # [bw, 27]
        A = B.reshape(bw // 16, 16, g.k).transpose(2, 1, 0)  # [27,16,bw/16]
        Bp = np.zeros((32, 16, bw // 16), np.int16)
        Bp[:g.k] = A
        for gi in range(4):
            cols.append(Bp[gi * 8:(gi + 1) * 8].reshape(128, bw // 16))
    return np.concatenate(cols, 1)                        # [128, at*4*ncol]


def _wrap2(iv, g):
    """conv2 idx block for one chunk: iv [c_chunk, 27] -> [128, ct*7*ncol]."""
    til = g.til
    ncol = til // 16
    cols = []
    for blk in range((g.ct + 1) // 2):
        bt = min(2, g.ct - blk * 2)
        bw = bt * til
        B = iv[blk * 2 * til: blk * 2 * til + bw]
        A = B.reshape(bw // 16, 16, g.k).transpose(2, 1, 0)  # [27,16,bw/16]
        Bp = np.zeros((28, 16, bw // 16), np.int16)
        Bp[:g.k] = A
        C = Bp.reshape(7, 4, 1, 16, bw // 16)
        D = np.broadcast_to(C, (7, 4, 2, 16, bw // 16))
        for gi in range(7):
            cols.append(np.ascontiguousarray(D[gi]).reshape(128, bw // 16))
    return np.concatenate(cols, 1)                        # [128, ct*7*ncol]


def prepare_inputs(g: Geo, feats, camera_cond, W1, g1, be1, W2, g2, be2,
                   Wc, bc, Wr, br, nbr, batch_idx):
    n = g.n
    til = g.til

    pos = np.arange(n, dtype=np.int64)
    dmax = 0
    for t in range(g.k):
        col = nbr[:, t].astype(np.int64)
        v = col >= 0
        if v.any():
            dmax = max(dmax, int(np.abs(col[v] - pos[v]).max()))
    assert dmax <= g.m, f"neighbor margin {dmax} exceeds {g.m}"

    fpg = _pack_pairs(feats)                      # [16, n]

    # weights
    W1b = np.asarray(W1, F32)
    wpk1 = np.zeros((128, 8 * 64), BF16)
    for gi in range(4):
        for q in range(G1_TAPS[gi]):
            t = gi * 8 + q
            for par in range(2):
                cb = (gi * 2 + par) * 64
                # rows q*16+j = W1[t, 2j+par, PERM1[m]]
                wpk1[q * 16:(q + 1) * 16, cb:cb + 64] = (
                    W1b[t, par::2, :][:, PERM1].astype(BF16))
    W2b = np.asarray(W2, F32)
    wpk2 = np.zeros((128, 14 * 64), BF16)
    for gi in range(7):
        for u in range(G2_TAPS[gi]):
            t = gi * 4 + u
            for par in range(2):
                cb = (gi * 2 + par) * 64
                wpk2[u * 32:(u + 1) * 32, cb:cb + 64] = (
                    W2b[t, par::2, :].astype(BF16))
    Wrb = np.asarray(Wr, F32)
    wrt = np.zeros((16, 128), BF16)
    wrt[:, 0:64] = Wrb[0::2, :].astype(BF16)
    wrt[:, 64:128] = Wrb[1::2, :].astype(BF16)
    brv = np.asarray(br, F32).reshape(64, 1)

    Wcn = np.asarray(Wc, F32)
    bcn = np.asarray(bc, F32)
    wcs = np.concatenate([Wcn[:, 0:64], bcn[None, 0:64]], 0).astype(F32)
    wcsh = np.concatenate([Wcn[:, 64:128], bcn[None, 64:128]], 0).astype(F32)
    camt = np.concatenate([np.asarray(camera_cond, F32).T,
                           np.ones((1, 4), F32)], 0)

    gb1 = np.stack([np.asarray(g1, F32)[PERM1],
                    np.asarray(be1, F32)[PERM1]], 1)
    gb2 = np.stack([np.asarray(g2, F32), np.asarray(be2, F32)], 1)

    nbr64 = nbr.astype(np.int64)
    valid_all = nbr64 >= 0

    in_maps = []
    for c in range(g.n_cores):
        own0 = c * g.own
        win0 = own0 - g.m
        f0 = own0 - 2 * g.m

        # feats window, replicated 8x along partitions for the 8-tap gathers
        fpc = np.zeros((16, g.fwinp), np.uint32)
        lo = max(f0, 0)
        hi = min(f0 + g.fwinp, n)
        if hi > lo:
            fpc[:, lo - f0:hi - f0] = fpg[:, lo:hi]

        # conv1 indices
        Gw = win0 + np.arange(g.win)
        inb = (Gw >= 0) & (Gw < n)
        Gc = np.clip(Gw, 0, n - 1)
        src = np.where((inb[:, None]) & valid_all[Gc], nbr64[Gc], -1)
        blocks = []
        for j in range(g.nch_a):
            rows = slice(j * g.a_chunk, (j + 1) * g.a_chunk)
            sr = src[rows]
            loc = sr - (f0 + j * g.a_chunk)
            iv = np.where(sr >= 0, loc, g.src_a - 1)
            assert iv.min() >= 0 and iv.max() <= g.src_a - 1
            blocks.append(_wrap1(iv.astype(np.int16), g))
        idx1 = np.concatenate(blocks, 1)

        # conv2 indices
        Go = own0 + np.arange(g.own_p)
        inb2 = Go < own0 + g.own
        Gc2 = np.clip(Go, 0, n - 1)
        src2 = np.where((inb2[:, None]) & valid_all[Gc2], nbr64[Gc2], -1)
        blocks2 = []
        for kk in range(g.nch_c):
            rows = slice(kk * g.c_chunk, (kk + 1) * g.c_chunk)
            sr = src2[rows]
            loc = sr - (win0 + kk * g.c_chunk)
            iv = np.where(sr >= 0, loc, g.src_c - 1)
            assert iv.min() >= 0 and iv.max() <= g.src_c - 1
            blocks2.append(_wrap2(iv.astype(np.int16), g))
        idx2 = np.concatenate(blocks2, 1)

        b = int(batch_idx[own0])
        bsel = np.zeros((64, 4), F32)
        bsel[:, b] = 1.0

        wbuf = np.zeros(WB_W, np.uint32)

        def put(name, arr_u32):
            off, rows, cols = WB_L[name]
            assert arr_u32.shape == (rows, cols), (name, arr_u32.shape)
            wbuf[off:off + rows * cols] = arr_u32.reshape(-1)

        put("w1", np.ascontiguousarray(wpk1).view(np.uint32))
        put("w2", np.ascontiguousarray(wpk2).view(np.uint32))
        put("wr", np.ascontiguousarray(wrt).view(np.uint32))
        for nm, arr in (("wcs", wcs), ("wcsh", wcsh), ("camt", camt)):
            put(nm + "a", np.ascontiguousarray(arr[0:128]).view(np.uint32))
            put(nm + "b", np.ascontiguousarray(arr[128:256]).view(np.uint32))
            put(nm + "c", np.ascontiguousarray(arr[256:257]).view(np.uint32))
        put("bsel", bsel.view(np.uint32))
        put("gb1", np.ascontiguousarray(gb1).view(np.uint32))
        put("gb2", np.ascontiguousarray(gb2).view(np.uint32))
        put("brv", brv.view(np.uint32))

        in_maps.append({"fp": fpc, "idx1": idx1, "idx2": idx2,
                        "wb": wbuf.reshape(1, WB_W)})
    return in_maps


# ---------------------------------------------------------------------------
# Entry point
# ---------------------------------------------------------------------------

_NC_CACHE = {}


def _get_module(g: Geo):
    if g not in _NC_CACHE:
        _NC_CACHE[g] = build_module(g)
    return _NC_CACHE[g]


def kernel(**inputs) -> np.ndarray:
    g = GEO_FULL
    nc = _get_module(g)
    args = {k: np.asarray(v) for k, v in inputs.items()}
    in_maps = prepare_inputs(
        g, args["feats"], args["camera_cond"], args["W1"], args["g1"],
        args["be1"], args["W2"], args["g2"], args["be2"], args["Wc"],
        args["bc"], args["Wr"], args["br"], args["nbr"], args["batch_idx"])
    res = run_bass_kernel_spmd(nc, in_maps, core_ids=list(range(g.n_cores)))
    out = np.concatenate(
        [res.results[c]["y"].T.astype(np.float32)
         for c in range(g.n_cores)], 0)
    return out

